# revision 96
# baseline (speedup 1.0000x reference)
"""Trainium2 Bass kernel for decomposed-rel-pos attention (B=4, H=W=32, DIM=768, HEADS=12).

Sharding: 48 (batch, head) pairs -> 8 cores x 6 heads (core c: batch c//2,
heads (c%2)*6 .. +6). All-bf16 dataflow (PE streams 1 col/cycle for bf16 and
f32r alike, but bf16 halves the serialized DMA-load bytes — HWDGE issue and
the DMA engines are both exclusive devices, so load time is serial). Inputs
are host-folded into SBUF-shaped DRAM tensors so each consumer is one large
contiguous DMA, issued in consumption-priority order on the SP queue. Bias is
folded into the S matmul as extra contraction rows (0/1 expander matrix
DMA-preloaded once into two persistent rhs_c buffers), softmax runs without
max-subtraction, row-sums come from a ones-column appended to V, and the
half-head projection partials are summed on host (bf16 output DMAs, paired
two token-tiles per transfer to halve HWDGE issue serialization).

Scheduling: steady state is PE-paced (~10.7us/head); ACT runs ONLY exp
(48 x [128,1024] = the would-be pacer) plus the k-copy, injected as column
halves mid-head where PE density absorbs the ACT-queue block. Per-head
pipeline: next head's qk projection injected in chunks at kb=1,2 of the
current S/exp ladder, q-scale + k-copy at kb=3, and both rel-pos tables at
kb=5 into a SINGLE merged PSUM tile (rows 0:32 rel_h, 32:64 rel_w) so the
head boundary costs one ladder slot and S(h+1,0) issues right at exp(h,7).
The V projection fills head 0's ladder (plus B0,B1 during the load tail).
After AV(7), the attn-out and denominator rows are copied PSUM->SBUF on DVE
immediately (releases the AV accumulator before AV(h+1,0) needs the slot);
the recip -> Pool partition-broadcast -> mul normalize then runs off the
critical path. The denominator must reach partition 0 via a PLAIN copy: the
custom-DVE reciprocal mishandles nonzero input partition offsets on HW, and
partition offsets must be multiples of 32 (BIR). The last head normalizes in
column halves (denom halves via ACT, which is idle by then) so the final
projection's t=2 matmuls start on half 1; the projection prefetches two PSUM
accumulations before that. PE warm-up matmuls on zeros during the initial
DMA wait keep the p-state ramp (and the cost model's dispatch-time pricing)
at full clock, and a dummy exp preloads the ACT table.

Cost-model (TimelineSim) estimate: ~102.1us/core (baseline: 133.0us);
engine busy ~ PE 83us, ACT 62us, DVE 52us. End-to-end rel err vs the fp32
jax reference: ~4.5e-3 (bf16-dominated).
"""
from contextlib import ExitStack

import numpy as np
import ml_dtypes

import concourse.bass as bass
import concourse.bacc as bacc
import concourse.mybir as mybir
import concourse.tile as tile
from concourse.bass_utils import run_bass_kernel_spmd

B, H, W, DIM, HEADS = 4, 32, 32, 768, 12
HD = DIM // HEADS  # 64
N = H * W  # 1024
HPC = HEADS // 2  # heads per core = 6
NCORES = 8
F32 = mybir.dt.float32
BF16 = mybir.dt.bfloat16

_cache = {}


def build_program(reps=1, debug=False):
    nc = bacc.Bacc("TRN2", target_bir_lowering=False, debug=False,
                   enable_asserts=False, num_devices=NCORES)
    xtf = nc.dram_tensor("xtf", [128, 7 * N], BF16, kind="ExternalInput")
    wqkf = nc.dram_tensor("wqkf", [128, HPC * 896], BF16, kind="ExternalInput")
    wvf = nc.dram_tensor("wvf", [128, 7 * 390], BF16, kind="ExternalInput")
    wpf = nc.dram_tensor("wpf", [128, 3 * 768], BF16, kind="ExternalInput")
    tabf = nc.dram_tensor("tabf", [HD, 2 * N], BF16, kind="ExternalInput")
    ecf = nc.dram_tensor("ecf", [64, N], BF16, kind="ExternalInput")
    bqf = nc.dram_tensor("bqf", [128, HPC], mybir.dt.float32, kind="ExternalInput")
    out_d = nc.dram_tensor("out_part", [N, DIM], BF16, kind="ExternalOutput")
    dbg = None
    if debug:
        dbg = {
            "rhs20": nc.dram_tensor("dbg_rhs20", [128, N], BF16, kind="ExternalOutput"),
            "lhsT0": nc.dram_tensor("dbg_lhsT0", [128, N], BF16, kind="ExternalOutput"),
            "at0": nc.dram_tensor("dbg_at0", [128, N], BF16, kind="ExternalOutput"),
            "pav0": nc.dram_tensor("dbg_pav0", [65, N], F32, kind="ExternalOutput"),
            "v0": nc.dram_tensor("dbg_v0", [128, 390], BF16, kind="ExternalOutput"),
            "xt0": nc.dram_tensor("dbg_xt0", [128, 7 * N], BF16, kind="ExternalOutput"),
            "rb0": nc.dram_tensor("dbg_rb0", [64, N], F32, kind="ExternalOutput"),
            "pl0": nc.dram_tensor("dbg_pl0", [128, N], BF16, kind="ExternalOutput"),
            "plf0": nc.dram_tensor("dbg_plf0", [128, N], BF16, kind="ExternalOutput"),
            "plf1": nc.dram_tensor("dbg_plf1", [128, N], BF16, kind="ExternalOutput"),
            "plf2": nc.dram_tensor("dbg_plf2", [128, N], BF16, kind="ExternalOutput"),
        }

    with ExitStack() as ctx:
        tc = ctx.enter_context(tile.TileContext(nc))
        _body(nc, tc, ctx, xtf, wqkf, wvf, wpf, tabf, ecf, bqf, out_d, reps, dbg)
    nc.compile()
    return nc


def _body(nc, tc, ctx, xtf, wqkf, wvf, wpf, tabf, ecf, bqf, out_d, reps, dbg=None):
    persist = ctx.enter_context(tc.tile_pool(name="persist", bufs=1))
    comb = ctx.enter_context(tc.tile_pool(name="comb", bufs=2))
    attn_pool = ctx.enter_context(tc.tile_pool(name="attn", bufs=1))
    nav = ctx.enter_context(tc.tile_pool(name="nav", bufs=3))
    outp = ctx.enter_context(tc.tile_pool(name="outp", bufs=4))
    ps_s = ctx.enter_context(tc.tile_pool(name="ps_s", bufs=2, space="PSUM"))
    ps_o = ctx.enter_context(tc.tile_pool(name="ps_o", bufs=2, space="PSUM"))

    xt = persist.tile([128, 7 * N], BF16, tag="xt", name="xt")
    wqk_sb = [persist.tile([128, 896], BF16, tag=f"wqk{h}", name=f"wqk{h}")
              for h in range(HPC)]
    wv_sb = persist.tile([128, 7 * 390], BF16, tag="wv", name="wv")
    wp_sb = persist.tile([128, 3 * 768], BF16, tag="wp", name="wp")
    tab = persist.tile([HD, 2 * N], BF16, tag="tab", name="tab")
    rhs2 = [persist.tile([128, N], BF16, tag=f"rhs{i}", name=f"rhs{i}")
            for i in range(2)]
    bq_sb = persist.tile([128, HPC], mybir.dt.float32, tag="bq", name="bq_sb")
    v_sb = [persist.tile([128, 390], BF16, tag=f"v{m}", name=f"v{m}")
            for m in range(8)]
    plhs = [persist.tile([128, N], BF16, tag=f"pl{t}", name=f"pl{t}")
            for t in range(3)]

    # ---- input loads: one SP queue, priority order (HWDGE+DMA serialize).
    # xt in two half-column DMAs: the QK projection's first column half can
    # start after only half the x bytes have landed.
    q = nc.sync
    q.dma_start(xt[:, 0:1024], xtf[:, 0:1024])
    q.dma_start(wqk_sb[0][:], wqkf[:, 0:896])
    q.dma_start(xt[:, 1024:3072], xtf[:, 1024:3072])
    q.dma_start(xt[:, 3072:5120], xtf[:, 3072:5120])
    q.dma_start(xt[:, 5120:7168], xtf[:, 5120:7168])
    q.dma_start(tab[:], tabf[:])
    q.dma_start(bq_sb[:], bqf[:])
    q.dma_start(rhs2[0][64:128, :], ecf[:])
    q.dma_start(wv_sb[:], wvf[:])
    q.dma_start(rhs2[1][64:128, :], ecf[:])
    for h in range(1, HPC):
        q.dma_start(wqk_sb[h][:], wqkf[:, h * 896:(h + 1) * 896])
    q.dma_start(wp_sb[:], wpf[:])

    # preload the Exp activation table while DMAs stream (otherwise the
    # 1.3us table load lands right before the first real exp)
    warm = persist.tile([1, 2], F32, tag="warm", name="warm")
    nc.gpsimd.memzero(warm[:])
    nc.scalar.activation(warm[0:1, 0:1], warm[0:1, 1:2],
                         mybir.ActivationFunctionType.Exp)

    # ---- phase helpers ----
    def A_chunk(h, pqk, kcs, halves=(0, 1)):
        # qk projection for head h, contraction tiles kcs (kc=6, the bias
        # row, is skipped: biases ride the scale / k-copy as [P,1] scalars)
        for half in halves:
            sl = slice(half * 512, half * 512 + 512)
            for kc in kcs:
                nc.tensor.matmul(
                    pqk[:, sl], wqk_sb[h][:, kc * 128:(kc + 1) * 128],
                    xt[:, kc * N + half * 512: kc * N + half * 512 + 512],
                    start=(kc == 0), stop=(kc == 5))

    def A_scale(h, pqk):
        lhsT_c = comb.tile([128, N], BF16, tag="lc", name="lhsT_c")
        nc.vector.tensor_scalar(lhsT_c[0:64, :], pqk[0:64, :],
                                bq_sb[0:64, h:h + 1], 0.125,
                                mybir.AluOpType.add, mybir.AluOpType.mult)
        return lhsT_c

    def k_copy(h, pqk):
        # Pool cannot read PSUM; ACT has slack (exp-only ~8.3us vs PE ~10.7us)
        # halves: shorter ACT-queue blocks between exps. Identity + bias AP
        # adds the k projection bias (kc=6 skipped in A_chunk).
        for sl in (slice(0, 512), slice(512, 1024)):
            nc.scalar.activation(rhs2[h % 2][0:64, sl], pqk[64:128, sl],
                                 mybir.ActivationFunctionType.Identity,
                                 bias=bq_sb[64:128, h:h + 1])

    def C_mms(lhsT_c):
        # both rel tables in ONE PSUM tile (rows 0:32 rel_h, 32:64 rel_w) so
        # C(h+1) consumes a single ladder slot at the head boundary
        pc = ps_s.tile([64, N], F32, tag="s", name="pc")
        for qh in range(32):
            sl = slice(qh * 32, qh * 32 + 32)
            nc.tensor.matmul(pc[0:32, sl], tab[:, sl], lhsT_c[0:64, sl],
                             start=True, stop=True)
        qT3 = lhsT_c[0:64, :].rearrange("p (a b) -> p b a", b=32)
        for qw in range(32):
            sl = slice(qw * 32, qw * 32 + 32)
            nc.tensor.matmul(pc[32:64, sl], tab[:, N + qw * 32:N + qw * 32 + 32],
                             qT3[:, qw, :], start=True, stop=True)
        return pc

    def C_copies(lhsT_c, pc, par=False):
        # half-1 copies first: they gate S(h,0); slot release gates S(h,1)
        prw_v = pc[32:64, :].rearrange("p (a b) -> p b a", b=32)
        # par: ACT takes the rel_h copies (only safe while ACT is exp-free)
        e1 = nc.scalar if par else nc.vector
        (e1.copy if par else e1.tensor_copy)(lhsT_c[64:96, 0:512], pc[0:32, 0:512])
        nc.vector.tensor_copy(lhsT_c[96:128, 0:512], prw_v[:, 0:16, :])
        (e1.copy if par else e1.tensor_copy)(lhsT_c[64:96, 512:1024], pc[0:32, 512:1024])
        nc.vector.tensor_copy(lhsT_c[96:128, 512:1024], prw_v[:, 16:32, :])

    def B_unit(m):
        pv = ps_s.tile([128, 390], F32, tag="s", name="pv")
        for kc in range(7):
            nc.tensor.matmul(pv[:], xt[:, kc * N + m * 128: kc * N + (m + 1) * 128],
                             wv_sb[:, kc * 390:(kc + 1) * 390],
                             start=(kc == 0), stop=(kc == 6))
        nc.vector.tensor_copy(v_sb[m][:], pv[:])

    # ---- D ladder ----
    def D_phase(h, lhsT_c, extra, post_s7):
        po = ps_o.tile([65, N], F32, tag="o", name="po")
        attnT = [attn_pool.tile([128, N], BF16, tag=f"at{kb}", name=f"at{kb}")
                 for kb in range(8)]

        def S_unit(kb, split_exp=False):
            ps = ps_s.tile([128, N], F32, tag="s", name="s_ps")
            for half in range(2):
                sl = slice(half * 512, half * 512 + 512)
                nc.tensor.matmul(ps[:, sl], rhs2[h % 2][:, kb * 128:(kb + 1) * 128],
                                 lhsT_c[:, sl], start=True, stop=True)
            if split_exp:
                # halves: AV(kb)-h1 and the normalize h1 chain start earlier
                for half in range(2):
                    sl = slice(half * 512, half * 512 + 512)
                    nc.scalar.activation(attnT[kb][:, sl], ps[:, sl],
                                         mybir.ActivationFunctionType.Exp)
            else:
                nc.scalar.activation(attnT[kb][:], ps[:],
                                     mybir.ActivationFunctionType.Exp)

        def AV_unit(kb):
            for half in range(2):
                sl = slice(half * 512, half * 512 + 512)
                nc.tensor.matmul(po[:, sl], v_sb[kb][:, h * 65:(h + 1) * 65],
                                 attnT[kb][:, sl], start=(kb == 0), stop=(kb == 7))

        for kb in range(8):
            S_unit(kb)
            if dbg is not None and h == 0 and kb == 0:
                nc.sync.dma_start(dbg["at0"][:], attnT[0][:])
            if dbg is not None and h == 0 and kb == 3:
                nc.sync.dma_start(dbg["v0"][:], v_sb[0][:])
            if kb in extra:
                for f in extra[kb]:
                    f()
            if kb >= 3:
                AV_unit(kb - 3)
        if post_s7 is not None:
            post_s7()

        def tail(last=False):
            AV_unit(5)
            AV_unit(6)
            AV_unit(7)
            t = plhs[h // 2][(h % 2) * 64:(h % 2) * 64 + 64, :]
            # po row 64 = softmax denominator (ones-column of V), rows 0:64 =
            # attention output. Copy PSUM->SBUF promptly to free the AV
            # accumulator; normalize runs off the critical path. The denom
            # row must reach partition 0 via a PLAIN copy (the custom-DVE
            # reciprocal mishandles nonzero input partition offsets on HW).
            if not last:
                dn = nav.tile([1, N], F32, tag="dn", name="dn")
                nc.vector.tensor_copy(dn[:], po[64:65, :])
                pav = nav.tile([64, N], F32, tag="pav", name="pav")
                nc.vector.tensor_copy(pav[:], po[0:64, :])
                if dbg is not None and h == 0:
                    nc.sync.dma_start(dbg["pav0"][:], pav[:])
                recip = nav.tile([1, N], F32, tag="rc", name="recip")
                nc.vector.reciprocal_approx_fast(out=recip[:], in_=dn[:])
                rb = nav.tile([64, N], F32, tag="rb", name="rb")
                nc.gpsimd.partition_broadcast(rb[:], recip[:])
                nc.vector.tensor_mul(t[:], pav[:], rb[:])
                if dbg is not None and h == 0:
                    nc.sync.dma_start(dbg["rb0"][:], rb[:])
                    nc.sync.dma_start(dbg["pl0"][:], plhs[0][:])
            else:
                # last head: denom row via ACT (free now); one full pav copy
                # releases po promptly (it gates the projection PSUM slots);
                # recip/bcast/mul per column half so t=2 matmuls start early
                # fully halved: ACT copies the denom halves, DVE runs
                # [recip-h, pav-h, mul-h] per half so mul-h1 (which gates the
                # projection's t=2 matmuls) completes as early as possible
                dn = nav.tile([1, N], F32, tag="dn", name="dn")
                pav = nav.tile([64, N], F32, tag="pav", name="pav")
                for half in range(2):
                    sl = slice(half * 512, half * 512 + 512)
                    nc.scalar.copy(dn[:, sl], po[64:65, sl])
                    recip = nav.tile([1, 512], F32, tag=f"rch{half}", name="reciph")
                    nc.vector.reciprocal_approx_fast(out=recip[:], in_=dn[:, sl])
                    rb = nav.tile([64, 512], F32, tag=f"rbh{half}", name="rbh")
                    nc.gpsimd.partition_broadcast(rb[:], recip[:])
                    nc.vector.tensor_copy(pav[:, sl], po[0:64, sl])
                    nc.vector.tensor_mul(t[:, sl], pav[:, sl], rb[:])
        return tail

    # ---- phase E: projection ----
    pps = {}

    def proj_alloc(m):
        pool, tg = (ps_s, "s") if m % 2 == 0 else (ps_o, "o")
        pps[m] = pool.tile([128, 768], F32, tag=tg, name="pp")

    def proj_mms(m, ts):
        pp = pps[m]
        for t in ts:
            for n0, nw in ((0, 512), (512, 256)):
                nc.tensor.matmul(pp[:, n0:n0 + nw],
                                 plhs[t][:, m * 128:(m + 1) * 128],
                                 wp_sb[:, t * 768 + n0: t * 768 + n0 + nw],
                                 start=(t == 0), stop=(t == 2))

    osb_pair = {}

    def proj_out(m):
        # m 0..5: copies per m (ACT/DVE alternating) into a paired staging
        # tile, ONE DMA per (even, odd) pair — halves serialized HWDGE issue.
        # m 6,7: single DMAs so the final chain after the last matmul is short
        if m >= 6:
            osb = outp.tile([128, DIM], BF16, tag="osbs", name="osbs")
            if m % 2 == 0:
                nc.scalar.copy(osb[:], pps[m][:])
            else:
                nc.vector.tensor_copy(osb[:], pps[m][:])
            eng = nc.sync if m % 2 == 0 else nc.scalar
            eng.dma_start(out_d[m * 128:(m + 1) * 128, :], osb[:])
        elif m % 2 == 0:
            osb = outp.tile([128, 2 * DIM], BF16, tag="osb", name="osb")
            osb_pair[m // 2] = osb
            nc.scalar.copy(osb[:, 0:DIM], pps[m][:])
        else:
            osb = osb_pair[m // 2]
            nc.vector.tensor_copy(osb[:, DIM:2 * DIM], pps[m][:])
            dview = out_d[(m - 1) * 128:(m + 1) * 128, :].rearrange(
                "(j p) d -> p j d", j=2)
            sview = osb[:].rearrange("p (j d) -> p j d", j=2)
            eng = nc.sync if m % 4 == 1 else nc.scalar
            eng.dma_start(dview, sview)

    # ---- main schedule ----
    for _rep in range(reps):
        pqk = ps_o.tile([128, N], F32, tag="o", name="pqk")
        if _rep == 0:
            # warm-up matmuls on zeros while input DMAs stream: the PE
            # p-state ramps with continuous execution, so the first real
            # matmuls are priced at full clock. pqk is reset by A's start=True.
            wu = persist.tile([64, 512], BF16, tag="wu", name="wu")
            nc.vector.memzero(wu[:])
            for _ in range(8):
                nc.tensor.matmul(pqk[0:64, 0:512], wu[:, 0:64], wu[:],
                                 start=True, stop=True)
        A_chunk(0, pqk, range(6))
        lhsT_c = A_scale(0, pqk)
        k_copy(0, pqk)
        pc0 = C_mms(lhsT_c)
        if _rep == 0:
            B_unit(0)  # fills PE while C-copies run on DVE
            B_unit(1)
        C_copies(lhsT_c, pc0)
        if dbg is not None:
            nc.sync.dma_start(dbg["xt0"][:], xt[:])
            nc.sync.dma_start(dbg["rhs20"][:], rhs2[0][:])
            nc.sync.dma_start(dbg["lhsT0"][:], lhsT_c[:])

        nxt = {}
        for h in range(HPC):
            extra = {}
            if h + 1 < HPC:
                def inj_a1(h=h):
                    nxt["pqk"] = ps_o.tile([128, N], F32, tag="o", name="pqk")
                    A_chunk(h + 1, nxt["pqk"], range(0, 3))

                def inj_a2(h=h):
                    A_chunk(h + 1, nxt["pqk"], range(3, 6))

                def inj_scale(h=h):
                    nxt["lhsT_c"] = A_scale(h + 1, nxt["pqk"])
                    k_copy(h + 1, nxt["pqk"])

                def inj_c(h=h):
                    pc = C_mms(nxt["lhsT_c"])
                    C_copies(nxt["lhsT_c"], pc)

                extra = {1: [inj_a1], 2: [inj_a2], 3: [inj_scale], 5: [inj_c]}
            if h == 0 and _rep == 0:
                base = dict(extra)
                for kb in range(6):
                    def mk(kb=kb):
                        return lambda: B_unit(kb + 2)
                    extra.setdefault(kb, [])
                    if kb in base:
                        extra[kb] = list(base[kb]) + [mk()]
                    else:
                        extra[kb] = [mk()]

            post_s7 = None
            if h == HPC - 1 and _rep == reps - 1:
                def post_s7():
                    proj_alloc(0)
                    proj_mms(0, (0, 1))
                    proj_alloc(1)
                    proj_mms(1, (0, 1))

            tail = D_phase(h, lhsT_c, extra, post_s7)
            if h + 1 < HPC:
                tail()
                lhsT_c = nxt["lhsT_c"]
            else:
                tail(last=True)

    if dbg is not None:
        for t in range(3):
            nc.sync.dma_start(dbg[f"plf{t}"][:], plhs[t][:])
    # E tail: t=2 for prefetched m, then the rest
    proj_alloc(2)
    proj_mms(2, (0, 1))
    for m in range(3):
        proj_mms(m, (2,))
        proj_out(m)
    proj_alloc(3)
    proj_mms(3, (0, 1))
    proj_mms(3, (2,))
    proj_out(3)
    for m in range(4, 8):
        proj_alloc(m)
        proj_mms(m, (0, 1, 2))
        proj_out(m)


def _host_prep(x, qkv_w, qkv_b, proj_w, proj_b, rel_pos_h, rel_pos_w):
    bf16 = ml_dtypes.bfloat16
    idx_h = np.arange(H)[:, None] - np.arange(H)[None, :] + (H - 1)
    idx_w = np.arange(W)[:, None] - np.arange(W)[None, :] + (W - 1)
    Rh = rel_pos_h[idx_h]  # [qh, kh, c]
    Rw = rel_pos_w[idx_w]  # [qw, kw, c]
    rhT8 = (8.0 * Rh).transpose(2, 0, 1).reshape(HD, H * H)
    rwT8 = (8.0 * Rw).transpose(2, 0, 1).reshape(HD, W * W)
    tabf = np.ascontiguousarray(
        np.concatenate([rhT8, rwT8], axis=1)).astype(bf16)
    kt = np.arange(N)
    ec = np.zeros((64, N), np.float32)
    ec[:32] = (np.arange(32)[:, None] == (kt // 32)[None, :])
    ec[32:] = (np.arange(32)[:, None] == (kt % 32)[None, :])
    ecf = ec.astype(bf16)

    in_maps = []
    for core in range(NCORES):
        b = core // 2
        h0 = (core % 2) * HPC
        xT = x[b].reshape(N, DIM).T  # [768, 1024]
        xtf = np.zeros((128, 7 * N), np.float32)
        for kc in range(6):
            xtf[:, kc * N:(kc + 1) * N] = xT[kc * 128:(kc + 1) * 128]
        xtf[0, 6 * N:7 * N] = 1.0

        wqkf = np.zeros((128, HPC * 896), np.float32)
        wvx = np.zeros((769, 7 * 0 + 390), np.float32)
        wpm = np.zeros((HPC * HD, DIM), np.float32)
        for h in range(HPC):
            g = h0 + h
            blk = np.zeros((769, 128), np.float32)
            blk[:DIM, 0:64] = qkv_w[g * HD:(g + 1) * HD].T
            blk[DIM, 0:64] = qkv_b[g * HD:(g + 1) * HD]
            blk[:DIM, 64:128] = qkv_w[DIM + g * HD:DIM + (g + 1) * HD].T
            blk[DIM, 64:128] = qkv_b[DIM + g * HD:DIM + (g + 1) * HD]
            for kc in range(6):
                wqkf[:, h * 896 + kc * 128: h * 896 + (kc + 1) * 128] = \
                    blk[kc * 128:(kc + 1) * 128]
            wqkf[0, h * 896 + 768: h * 896 + 896] = blk[DIM]

            wvx[:DIM, h * 65:h * 65 + 64] = \
                qkv_w[2 * DIM + g * HD:2 * DIM + (g + 1) * HD].T
            wvx[DIM, h * 65:h * 65 + 64] = \
                qkv_b[2 * DIM + g * HD:2 * DIM + (g + 1) * HD]
            wvx[DIM, h * 65 + 64] = 1.0
            wpm[h * HD:(h + 1) * HD, :] = proj_w[:, g * HD:(g + 1) * HD].T

        wvf = np.zeros((128, 7 * 390), np.float32)
        for kc in range(6):
            wvf[:, kc * 390:(kc + 1) * 390] = wvx[kc * 128:(kc + 1) * 128]
        wvf[0, 6 * 390:7 * 390] = wvx[DIM]

        wpf = np.zeros((128, 3 * 768), np.float32)
        for t in range(3):
            wpf[:, t * 768:(t + 1) * 768] = wpm[t * 128:(t + 1) * 128]

        bqm = np.zeros((128, HPC), np.float32)
        for h in range(HPC):
            g = h0 + h
            bqm[0:64, h] = qkv_b[g * HD:(g + 1) * HD]
            bqm[64:128, h] = qkv_b[DIM + g * HD:DIM + (g + 1) * HD]
        in_maps.append({
            "xtf": xtf.astype(bf16), "wqkf": wqkf.astype(bf16),
            "wvf": wvf.astype(bf16), "wpf": wpf.astype(bf16),
            "tabf": tabf, "ecf": ecf, "bqf": bqm,
        })
    return in_maps


def kernel(x, qkv_w, qkv_b, proj_w, proj_b, rel_pos_h, rel_pos_w, _trace=False):
    x = np.asarray(x, np.float32)
    qkv_w = np.asarray(qkv_w, np.float32)
    qkv_b = np.asarray(qkv_b, np.float32)
    proj_w = np.asarray(proj_w, np.float32)
    proj_b = np.asarray(proj_b, np.float32)
    rel_pos_h = np.asarray(rel_pos_h, np.float32)
    rel_pos_w = np.asarray(rel_pos_w, np.float32)

    in_maps = _host_prep(x, qkv_w, qkv_b, proj_w, proj_b, rel_pos_h, rel_pos_w)
    if "nc" not in _cache:
        _cache["nc"] = build_program()
    nc = _cache["nc"]
    res = run_bass_kernel_spmd(nc, in_maps, core_ids=list(range(NCORES)),
                               trace=_trace)
    parts = [r["out_part"].astype(np.float32) for r in res.results]
    out = np.zeros((B, N, DIM), np.float32)
    for b in range(B):
        out[b] = parts[2 * b] + parts[2 * b + 1] + proj_b
    if _trace:
        kernel.last_results = res
    return out.reshape(B, H, W, DIM)


# revision 99
# speedup vs baseline: 1.0004x; 1.0004x over previous
"""Trainium2 Bass kernel for decomposed-rel-pos attention (B=4, H=W=32, DIM=768, HEADS=12).

Sharding: 48 (batch, head) pairs -> 8 cores x 6 heads (core c: batch c//2,
heads (c%2)*6 .. +6). All-bf16 dataflow (PE streams 1 col/cycle for bf16 and
f32r alike, but bf16 halves the serialized DMA-load bytes — HWDGE issue and
the DMA engines are both exclusive devices, so load time is serial). Inputs
are host-folded into SBUF-shaped DRAM tensors so each consumer is one large
contiguous DMA, issued in consumption-priority order on the SP queue. Bias is
folded into the S matmul as extra contraction rows (0/1 expander matrix
DMA-preloaded once into two persistent rhs_c buffers), softmax runs without
max-subtraction, row-sums come from a ones-column appended to V, and the
half-head projection partials are summed on host (bf16 output DMAs, paired
two token-tiles per transfer to halve HWDGE issue serialization).

Scheduling: steady state is PE-paced (~10.7us/head); ACT runs ONLY exp
(48 x [128,1024] = the would-be pacer) plus the k-copy, injected as column
halves mid-head where PE density absorbs the ACT-queue block. Per-head
pipeline: next head's qk projection injected in chunks at kb=1,2 of the
current S/exp ladder, q-scale + k-copy at kb=3, and both rel-pos tables at
kb=5 into a SINGLE merged PSUM tile (rows 0:32 rel_h, 32:64 rel_w) so the
head boundary costs one ladder slot and S(h+1,0) issues right at exp(h,7).
The V projection fills head 0's ladder (plus B0,B1 during the load tail).
After AV(7), the attn-out and denominator rows are copied PSUM->SBUF on DVE
immediately (releases the AV accumulator before AV(h+1,0) needs the slot);
the recip -> Pool partition-broadcast -> mul normalize then runs off the
critical path. The denominator must reach partition 0 via a PLAIN copy: the
custom-DVE reciprocal mishandles nonzero input partition offsets on HW, and
partition offsets must be multiples of 32 (BIR). The last head normalizes in
column halves (denom halves via ACT, which is idle by then) so the final
projection's t=2 matmuls start on half 1; the projection prefetches two PSUM
accumulations before that. PE warm-up matmuls on zeros during the initial
DMA wait keep the p-state ramp (and the cost model's dispatch-time pricing)
at full clock, and a dummy exp preloads the ACT table.

Cost-model (TimelineSim) estimate: ~102.1us/core (baseline: 133.0us);
engine busy ~ PE 83us, ACT 62us, DVE 52us. End-to-end rel err vs the fp32
jax reference: ~4.5e-3 (bf16-dominated).
"""
from contextlib import ExitStack

import numpy as np
import ml_dtypes

import concourse.bass as bass
import concourse.bacc as bacc
import concourse.mybir as mybir
import concourse.tile as tile
from concourse.bass_utils import run_bass_kernel_spmd

B, H, W, DIM, HEADS = 4, 32, 32, 768, 12
HD = DIM // HEADS  # 64
N = H * W  # 1024
HPC = HEADS // 2  # heads per core = 6
NCORES = 8
F32 = mybir.dt.float32
BF16 = mybir.dt.bfloat16

_cache = {}


def build_program(reps=1, debug=False):
    nc = bacc.Bacc("TRN2", target_bir_lowering=False, debug=False,
                   enable_asserts=False, num_devices=NCORES)
    xtf = nc.dram_tensor("xtf", [128, 7 * N], BF16, kind="ExternalInput")
    wqkf = nc.dram_tensor("wqkf", [128, HPC * 896], BF16, kind="ExternalInput")
    wvf = nc.dram_tensor("wvf", [128, 7 * 390], BF16, kind="ExternalInput")
    wpf = nc.dram_tensor("wpf", [128, 3 * 768], BF16, kind="ExternalInput")
    tabf = nc.dram_tensor("tabf", [HD, 2 * N], BF16, kind="ExternalInput")
    ecf = nc.dram_tensor("ecf", [64, N], BF16, kind="ExternalInput")
    bqf = nc.dram_tensor("bqf", [128, HPC], mybir.dt.float32, kind="ExternalInput")
    out_d = nc.dram_tensor("out_part", [N, DIM], BF16, kind="ExternalOutput")
    dbg = None
    if debug:
        dbg = {
            "rhs20": nc.dram_tensor("dbg_rhs20", [128, N], BF16, kind="ExternalOutput"),
            "lhsT0": nc.dram_tensor("dbg_lhsT0", [128, N], BF16, kind="ExternalOutput"),
            "at0": nc.dram_tensor("dbg_at0", [128, N], BF16, kind="ExternalOutput"),
            "pav0": nc.dram_tensor("dbg_pav0", [65, N], F32, kind="ExternalOutput"),
            "v0": nc.dram_tensor("dbg_v0", [128, 390], BF16, kind="ExternalOutput"),
            "xt0": nc.dram_tensor("dbg_xt0", [128, 7 * N], BF16, kind="ExternalOutput"),
            "rb0": nc.dram_tensor("dbg_rb0", [64, N], F32, kind="ExternalOutput"),
            "pl0": nc.dram_tensor("dbg_pl0", [128, N], BF16, kind="ExternalOutput"),
            "plf0": nc.dram_tensor("dbg_plf0", [128, N], BF16, kind="ExternalOutput"),
            "plf1": nc.dram_tensor("dbg_plf1", [128, N], BF16, kind="ExternalOutput"),
            "plf2": nc.dram_tensor("dbg_plf2", [128, N], BF16, kind="ExternalOutput"),
        }

    with ExitStack() as ctx:
        tc = ctx.enter_context(tile.TileContext(nc))
        _body(nc, tc, ctx, xtf, wqkf, wvf, wpf, tabf, ecf, bqf, out_d, reps, dbg)
    nc.compile()
    return nc


def _body(nc, tc, ctx, xtf, wqkf, wvf, wpf, tabf, ecf, bqf, out_d, reps, dbg=None):
    persist = ctx.enter_context(tc.tile_pool(name="persist", bufs=1))
    comb = ctx.enter_context(tc.tile_pool(name="comb", bufs=2))
    attn_pool = ctx.enter_context(tc.tile_pool(name="attn", bufs=1))
    nav = ctx.enter_context(tc.tile_pool(name="nav", bufs=3))
    outp = ctx.enter_context(tc.tile_pool(name="outp", bufs=4))
    ps_s = ctx.enter_context(tc.tile_pool(name="ps_s", bufs=2, space="PSUM"))
    ps_o = ctx.enter_context(tc.tile_pool(name="ps_o", bufs=2, space="PSUM"))

    xt = persist.tile([128, 7 * N], BF16, tag="xt", name="xt")
    wqk_sb = [persist.tile([128, 896], BF16, tag=f"wqk{h}", name=f"wqk{h}")
              for h in range(HPC)]
    wv_sb = persist.tile([128, 7 * 390], BF16, tag="wv", name="wv")
    wp_sb = persist.tile([128, 3 * 768], BF16, tag="wp", name="wp")
    tab = persist.tile([HD, 2 * N], BF16, tag="tab", name="tab")
    rhs2 = [persist.tile([128, N], BF16, tag=f"rhs{i}", name=f"rhs{i}")
            for i in range(2)]
    bq_sb = persist.tile([128, HPC], mybir.dt.float32, tag="bq", name="bq_sb")
    v_sb = [persist.tile([128, 390], BF16, tag=f"v{m}", name=f"v{m}")
            for m in range(8)]
    plhs = [persist.tile([128, N], BF16, tag=f"pl{t}", name=f"pl{t}")
            for t in range(3)]

    # ---- input loads: one SP queue, priority order (HWDGE+DMA serialize).
    # xt in two half-column DMAs: the QK projection's first column half can
    # start after only half the x bytes have landed.
    q = nc.sync
    q.dma_start(xt[:, 0:1024], xtf[:, 0:1024])
    q.dma_start(wqk_sb[0][:], wqkf[:, 0:896])
    q.dma_start(xt[:, 1024:3072], xtf[:, 1024:3072])
    q.dma_start(xt[:, 3072:5120], xtf[:, 3072:5120])
    q.dma_start(xt[:, 5120:7168], xtf[:, 5120:7168])
    q.dma_start(tab[:], tabf[:])
    q.dma_start(bq_sb[:], bqf[:])
    q.dma_start(rhs2[0][64:128, :], ecf[:])
    q.dma_start(wv_sb[:], wvf[:])
    q.dma_start(rhs2[1][64:128, :], ecf[:])
    for h in range(1, HPC):
        q.dma_start(wqk_sb[h][:], wqkf[:, h * 896:(h + 1) * 896])
    q.dma_start(wp_sb[:], wpf[:])

    # preload the Exp activation table while DMAs stream (otherwise the
    # 1.3us table load lands right before the first real exp)
    warm = persist.tile([1, 2], F32, tag="warm", name="warm")
    nc.gpsimd.memzero(warm[:])
    nc.scalar.activation(warm[0:1, 0:1], warm[0:1, 1:2],
                         mybir.ActivationFunctionType.Exp)

    # ---- phase helpers ----
    def A_chunk(h, pqk, kcs, halves=(0, 1)):
        # qk projection for head h, contraction tiles kcs (kc=6, the bias
        # row, is skipped: biases ride the scale / k-copy as [P,1] scalars)
        for half in halves:
            sl = slice(half * 512, half * 512 + 512)
            for kc in kcs:
                nc.tensor.matmul(
                    pqk[:, sl], wqk_sb[h][:, kc * 128:(kc + 1) * 128],
                    xt[:, kc * N + half * 512: kc * N + half * 512 + 512],
                    start=(kc == 0), stop=(kc == 5))

    def A_scale(h, pqk):
        lhsT_c = comb.tile([128, N], BF16, tag="lc", name="lhsT_c")
        nc.vector.tensor_scalar(lhsT_c[0:64, :], pqk[0:64, :],
                                bq_sb[0:64, h:h + 1], 0.125,
                                mybir.AluOpType.add, mybir.AluOpType.mult)
        return lhsT_c

    def k_copy(h, pqk):
        # Pool cannot read PSUM; ACT has slack (exp-only ~8.3us vs PE ~10.7us)
        # halves: shorter ACT-queue blocks between exps. Identity + bias AP
        # adds the k projection bias (kc=6 skipped in A_chunk).
        for sl in (slice(0, 512), slice(512, 1024)):
            nc.scalar.activation(rhs2[h % 2][0:64, sl], pqk[64:128, sl],
                                 mybir.ActivationFunctionType.Identity,
                                 bias=bq_sb[64:128, h:h + 1])

    def C_mms(lhsT_c):
        # both rel tables in ONE PSUM tile (rows 0:32 rel_h, 32:64 rel_w) so
        # C(h+1) consumes a single ladder slot at the head boundary
        pc = ps_s.tile([64, N], F32, tag="s", name="pc")
        for qh in range(32):
            sl = slice(qh * 32, qh * 32 + 32)
            nc.tensor.matmul(pc[0:32, sl], tab[:, sl], lhsT_c[0:64, sl],
                             start=True, stop=True)
        qT3 = lhsT_c[0:64, :].rearrange("p (a b) -> p b a", b=32)
        for qw in range(32):
            sl = slice(qw * 32, qw * 32 + 32)
            nc.tensor.matmul(pc[32:64, sl], tab[:, N + qw * 32:N + qw * 32 + 32],
                             qT3[:, qw, :], start=True, stop=True)
        return pc

    def C_copies(lhsT_c, pc, par=False):
        # half-1 copies first: they gate S(h,0); slot release gates S(h,1)
        prw_v = pc[32:64, :].rearrange("p (a b) -> p b a", b=32)
        # par: ACT takes the rel_h copies (only safe while ACT is exp-free)
        e1 = nc.scalar if par else nc.vector
        (e1.copy if par else e1.tensor_copy)(lhsT_c[64:96, 0:512], pc[0:32, 0:512])
        nc.vector.tensor_copy(lhsT_c[96:128, 0:512], prw_v[:, 0:16, :])
        (e1.copy if par else e1.tensor_copy)(lhsT_c[64:96, 512:1024], pc[0:32, 512:1024])
        nc.vector.tensor_copy(lhsT_c[96:128, 512:1024], prw_v[:, 16:32, :])

    def B_unit(m):
        pv = ps_s.tile([128, 390], F32, tag="s", name="pv")
        for kc in range(7):
            nc.tensor.matmul(pv[:], xt[:, kc * N + m * 128: kc * N + (m + 1) * 128],
                             wv_sb[:, kc * 390:(kc + 1) * 390],
                             start=(kc == 0), stop=(kc == 6))
        nc.vector.tensor_copy(v_sb[m][:], pv[:])

    # ---- D ladder ----
    def D_phase(h, lhsT_c, extra, post_s7):
        po = ps_o.tile([65, N], F32, tag="o", name="po")
        attnT = [attn_pool.tile([128, N], BF16, tag=f"at{kb}", name=f"at{kb}")
                 for kb in range(8)]

        def S_unit(kb, split_exp=False):
            ps = ps_s.tile([128, N], F32, tag="s", name="s_ps")
            for half in range(2):
                sl = slice(half * 512, half * 512 + 512)
                nc.tensor.matmul(ps[:, sl], rhs2[h % 2][:, kb * 128:(kb + 1) * 128],
                                 lhsT_c[:, sl], start=True, stop=True)
            if split_exp:
                # halves: AV(kb)-h1 and the normalize h1 chain start earlier
                for half in range(2):
                    sl = slice(half * 512, half * 512 + 512)
                    nc.scalar.activation(attnT[kb][:, sl], ps[:, sl],
                                         mybir.ActivationFunctionType.Exp)
            else:
                nc.scalar.activation(attnT[kb][:], ps[:],
                                     mybir.ActivationFunctionType.Exp)

        def AV_unit(kb):
            for half in range(2):
                sl = slice(half * 512, half * 512 + 512)
                nc.tensor.matmul(po[:, sl], v_sb[kb][:, h * 65:(h + 1) * 65],
                                 attnT[kb][:, sl], start=(kb == 0), stop=(kb == 7))

        for kb in range(8):
            S_unit(kb)
            if dbg is not None and h == 0 and kb == 0:
                nc.sync.dma_start(dbg["at0"][:], attnT[0][:])
            if dbg is not None and h == 0 and kb == 3:
                nc.sync.dma_start(dbg["v0"][:], v_sb[0][:])
            if kb in extra:
                for f in extra[kb]:
                    f()
            if kb >= 3:
                AV_unit(kb - 3)
        if post_s7 is not None:
            post_s7()

        def tail(last=False):
            AV_unit(5)
            AV_unit(6)
            AV_unit(7)
            t = plhs[h // 2][(h % 2) * 64:(h % 2) * 64 + 64, :]
            # po row 64 = softmax denominator (ones-column of V), rows 0:64 =
            # attention output. Copy PSUM->SBUF promptly to free the AV
            # accumulator; normalize runs off the critical path. The denom
            # row must reach partition 0 via a PLAIN copy (the custom-DVE
            # reciprocal mishandles nonzero input partition offsets on HW).
            if not last:
                dn = nav.tile([1, N], F32, tag="dn", name="dn")
                nc.vector.tensor_copy(dn[:], po[64:65, :])
                pav = nav.tile([64, N], F32, tag="pav", name="pav")
                nc.vector.tensor_copy(pav[:], po[0:64, :])
                if dbg is not None and h == 0:
                    nc.sync.dma_start(dbg["pav0"][:], pav[:])
                recip = nav.tile([1, N], F32, tag="rc", name="recip")
                nc.vector.reciprocal_approx_fast(out=recip[:], in_=dn[:])
                rb = nav.tile([64, N], F32, tag="rb", name="rb")
                nc.gpsimd.partition_broadcast(rb[:], recip[:])
                nc.vector.tensor_mul(t[:], pav[:], rb[:])
                if dbg is not None and h == 0:
                    nc.sync.dma_start(dbg["rb0"][:], rb[:])
                    nc.sync.dma_start(dbg["pl0"][:], plhs[0][:])
            else:
                # last head: denom row via ACT (free now); one full pav copy
                # releases po promptly (it gates the projection PSUM slots);
                # recip/bcast/mul per column half so t=2 matmuls start early
                # fully halved: ACT copies the denom halves, DVE runs
                # [recip-h, pav-h, mul-h] per half so mul-h1 (which gates the
                # projection's t=2 matmuls) completes as early as possible
                dn = nav.tile([1, N], F32, tag="dn", name="dn")
                pav = nav.tile([64, N], F32, tag="pav", name="pav")
                for half in range(2):
                    sl = slice(half * 512, half * 512 + 512)
                    nc.scalar.copy(dn[:, sl], po[64:65, sl])
                    recip = nav.tile([1, 512], F32, tag=f"rch{half}", name="reciph")
                    nc.vector.reciprocal_approx_fast(out=recip[:], in_=dn[:, sl])
                    rb = nav.tile([64, 512], F32, tag=f"rbh{half}", name="rbh")
                    nc.gpsimd.partition_broadcast(rb[:], recip[:])
                    nc.vector.tensor_copy(pav[:, sl], po[0:64, sl])
                    nc.vector.tensor_mul(t[:, sl], pav[:, sl], rb[:])
        return tail

    # ---- phase E: projection ----
    pps = {}

    def proj_alloc(m):
        pool, tg = (ps_s, "s") if m % 2 == 0 else (ps_o, "o")
        pps[m] = pool.tile([128, 768], F32, tag=tg, name="pp")

    def proj_mms(m, ts):
        pp = pps[m]
        for t in ts:
            for n0, nw in ((0, 512), (512, 256)):
                nc.tensor.matmul(pp[:, n0:n0 + nw],
                                 plhs[t][:, m * 128:(m + 1) * 128],
                                 wp_sb[:, t * 768 + n0: t * 768 + n0 + nw],
                                 start=(t == 0), stop=(t == 2))

    osb_pair = {}

    def proj_out(m):
        # m 0..5: copies per m (ACT/DVE alternating) into a paired staging
        # tile, ONE DMA per (even, odd) pair — halves serialized HWDGE issue.
        # m 6,7: single DMAs so the final chain after the last matmul is short
        if m >= 6:
            osb = outp.tile([128, DIM], BF16, tag="osbs", name="osbs")
            if m % 2 == 0:
                nc.vector.tensor_copy(osb[:], pps[m][:])
            else:
                nc.scalar.copy(osb[:], pps[m][:])
            eng = nc.sync if m % 2 == 0 else nc.scalar
            eng.dma_start(out_d[m * 128:(m + 1) * 128, :], osb[:])
        elif m % 2 == 0:
            osb = outp.tile([128, 2 * DIM], BF16, tag="osb", name="osb")
            osb_pair[m // 2] = osb
            nc.scalar.copy(osb[:, 0:DIM], pps[m][:])
        else:
            osb = osb_pair[m // 2]
            nc.vector.tensor_copy(osb[:, DIM:2 * DIM], pps[m][:])
            dview = out_d[(m - 1) * 128:(m + 1) * 128, :].rearrange(
                "(j p) d -> p j d", j=2)
            sview = osb[:].rearrange("p (j d) -> p j d", j=2)
            eng = nc.sync if m % 4 == 1 else nc.scalar
            eng.dma_start(dview, sview)

    # ---- main schedule ----
    for _rep in range(reps):
        pqk = ps_o.tile([128, N], F32, tag="o", name="pqk")
        if _rep == 0:
            # warm-up matmuls on zeros while input DMAs stream: the PE
            # p-state ramps with continuous execution, so the first real
            # matmuls are priced at full clock. pqk is reset by A's start=True.
            wu = persist.tile([64, 512], BF16, tag="wu", name="wu")
            nc.vector.memzero(wu[:])
            for _ in range(8):
                nc.tensor.matmul(pqk[0:64, 0:512], wu[:, 0:64], wu[:],
                                 start=True, stop=True)
        A_chunk(0, pqk, range(6))
        lhsT_c = A_scale(0, pqk)
        k_copy(0, pqk)
        pc0 = C_mms(lhsT_c)
        if _rep == 0:
            B_unit(0)  # fills PE while C-copies run on DVE
            B_unit(1)
        C_copies(lhsT_c, pc0)
        if dbg is not None:
            nc.sync.dma_start(dbg["xt0"][:], xt[:])
            nc.sync.dma_start(dbg["rhs20"][:], rhs2[0][:])
            nc.sync.dma_start(dbg["lhsT0"][:], lhsT_c[:])

        nxt = {}
        for h in range(HPC):
            extra = {}
            if h + 1 < HPC:
                def inj_a1(h=h):
                    nxt["pqk"] = ps_o.tile([128, N], F32, tag="o", name="pqk")
                    A_chunk(h + 1, nxt["pqk"], range(0, 3))

                def inj_a2(h=h):
                    A_chunk(h + 1, nxt["pqk"], range(3, 6))

                def inj_scale(h=h):
                    nxt["lhsT_c"] = A_scale(h + 1, nxt["pqk"])
                    k_copy(h + 1, nxt["pqk"])

                def inj_c(h=h):
                    pc = C_mms(nxt["lhsT_c"])
                    C_copies(nxt["lhsT_c"], pc)

                extra = {1: [inj_a1], 2: [inj_a2], 3: [inj_scale], 5: [inj_c]}
            if h == 0 and _rep == 0:
                base = dict(extra)
                for kb in range(6):
                    def mk(kb=kb):
                        return lambda: B_unit(kb + 2)
                    extra.setdefault(kb, [])
                    if kb in base:
                        extra[kb] = list(base[kb]) + [mk()]
                    else:
                        extra[kb] = [mk()]

            post_s7 = None
            if h == HPC - 1 and _rep == reps - 1:
                def post_s7():
                    proj_alloc(0)
                    proj_mms(0, (0, 1))
                    proj_alloc(1)
                    proj_mms(1, (0, 1))

            tail = D_phase(h, lhsT_c, extra, post_s7)
            if h + 1 < HPC:
                tail()
                lhsT_c = nxt["lhsT_c"]
            else:
                tail(last=True)

    if dbg is not None:
        for t in range(3):
            nc.sync.dma_start(dbg[f"plf{t}"][:], plhs[t][:])
    # E tail: t=2 for prefetched m, then the rest
    proj_alloc(2)
    proj_mms(2, (0, 1))
    for m in range(3):
        proj_mms(m, (2,))
        proj_out(m)
    proj_alloc(3)
    proj_mms(3, (0, 1))
    proj_mms(3, (2,))
    proj_out(3)
    for m in range(4, 8):
        proj_alloc(m)
        proj_mms(m, (0, 1, 2))
        proj_out(m)


def _host_prep(x, qkv_w, qkv_b, proj_w, proj_b, rel_pos_h, rel_pos_w):
    bf16 = ml_dtypes.bfloat16
    idx_h = np.arange(H)[:, None] - np.arange(H)[None, :] + (H - 1)
    idx_w = np.arange(W)[:, None] - np.arange(W)[None, :] + (W - 1)
    Rh = rel_pos_h[idx_h]  # [qh, kh, c]
    Rw = rel_pos_w[idx_w]  # [qw, kw, c]
    rhT8 = (8.0 * Rh).transpose(2, 0, 1).reshape(HD, H * H)
    rwT8 = (8.0 * Rw).transpose(2, 0, 1).reshape(HD, W * W)
    tabf = np.ascontiguousarray(
        np.concatenate([rhT8, rwT8], axis=1)).astype(bf16)
    kt = np.arange(N)
    ec = np.zeros((64, N), np.float32)
    ec[:32] = (np.arange(32)[:, None] == (kt // 32)[None, :])
    ec[32:] = (np.arange(32)[:, None] == (kt % 32)[None, :])
    ecf = ec.astype(bf16)

    in_maps = []
    for core in range(NCORES):
        b = core // 2
        h0 = (core % 2) * HPC
        xT = x[b].reshape(N, DIM).T  # [768, 1024]
        xtf = np.zeros((128, 7 * N), np.float32)
        for kc in range(6):
            xtf[:, kc * N:(kc + 1) * N] = xT[kc * 128:(kc + 1) * 128]
        xtf[0, 6 * N:7 * N] = 1.0

        wqkf = np.zeros((128, HPC * 896), np.float32)
        wvx = np.zeros((769, 7 * 0 + 390), np.float32)
        wpm = np.zeros((HPC * HD, DIM), np.float32)
        for h in range(HPC):
            g = h0 + h
            blk = np.zeros((769, 128), np.float32)
            blk[:DIM, 0:64] = qkv_w[g * HD:(g + 1) * HD].T
            blk[DIM, 0:64] = qkv_b[g * HD:(g + 1) * HD]
            blk[:DIM, 64:128] = qkv_w[DIM + g * HD:DIM + (g + 1) * HD].T
            blk[DIM, 64:128] = qkv_b[DIM + g * HD:DIM + (g + 1) * HD]
            for kc in range(6):
                wqkf[:, h * 896 + kc * 128: h * 896 + (kc + 1) * 128] = \
                    blk[kc * 128:(kc + 1) * 128]
            wqkf[0, h * 896 + 768: h * 896 + 896] = blk[DIM]

            wvx[:DIM, h * 65:h * 65 + 64] = \
                qkv_w[2 * DIM + g * HD:2 * DIM + (g + 1) * HD].T
            wvx[DIM, h * 65:h * 65 + 64] = \
                qkv_b[2 * DIM + g * HD:2 * DIM + (g + 1) * HD]
            wvx[DIM, h * 65 + 64] = 1.0
            wpm[h * HD:(h + 1) * HD, :] = proj_w[:, g * HD:(g + 1) * HD].T

        wvf = np.zeros((128, 7 * 390), np.float32)
        for kc in range(6):
            wvf[:, kc * 390:(kc + 1) * 390] = wvx[kc * 128:(kc + 1) * 128]
        wvf[0, 6 * 390:7 * 390] = wvx[DIM]

        wpf = np.zeros((128, 3 * 768), np.float32)
        for t in range(3):
            wpf[:, t * 768:(t + 1) * 768] = wpm[t * 128:(t + 1) * 128]

        bqm = np.zeros((128, HPC), np.float32)
        for h in range(HPC):
            g = h0 + h
            bqm[0:64, h] = qkv_b[g * HD:(g + 1) * HD]
            bqm[64:128, h] = qkv_b[DIM + g * HD:DIM + (g + 1) * HD]
        in_maps.append({
            "xtf": xtf.astype(bf16), "wqkf": wqkf.astype(bf16),
            "wvf": wvf.astype(bf16), "wpf": wpf.astype(bf16),
            "tabf": tabf, "ecf": ecf, "bqf": bqm,
        })
    return in_maps


def kernel(x, qkv_w, qkv_b, proj_w, proj_b, rel_pos_h, rel_pos_w, _trace=False):
    x = np.asarray(x, np.float32)
    qkv_w = np.asarray(qkv_w, np.float32)
    qkv_b = np.asarray(qkv_b, np.float32)
    proj_w = np.asarray(proj_w, np.float32)
    proj_b = np.asarray(proj_b, np.float32)
    rel_pos_h = np.asarray(rel_pos_h, np.float32)
    rel_pos_w = np.asarray(rel_pos_w, np.float32)

    in_maps = _host_prep(x, qkv_w, qkv_b, proj_w, proj_b, rel_pos_h, rel_pos_w)
    if "nc" not in _cache:
        _cache["nc"] = build_program()
    nc = _cache["nc"]
    res = run_bass_kernel_spmd(nc, in_maps, core_ids=list(range(NCORES)),
                               trace=_trace)
    parts = [r["out_part"].astype(np.float32) for r in res.results]
    out = np.zeros((B, N, DIM), np.float32)
    for b in range(B):
        out[b] = parts[2 * b] + parts[2 * b + 1] + proj_b
    if _trace:
        kernel.last_results = res
    return out.reshape(B, H, W, DIM)


# revision 100
# speedup vs baseline: 1.0011x; 1.0007x over previous
"""Trainium2 Bass kernel for decomposed-rel-pos attention (B=4, H=W=32, DIM=768, HEADS=12).

Sharding: 48 (batch, head) pairs -> 8 cores x 6 heads (core c: batch c//2,
heads (c%2)*6 .. +6). All-bf16 dataflow (PE streams 1 col/cycle for bf16 and
f32r alike, but bf16 halves the serialized DMA-load bytes — HWDGE issue and
the DMA engines are both exclusive devices, so load time is serial). Inputs
are host-folded into SBUF-shaped DRAM tensors so each consumer is one large
contiguous DMA, issued in consumption-priority order on the SP queue. Bias is
folded into the S matmul as extra contraction rows (0/1 expander matrix
DMA-preloaded once into two persistent rhs_c buffers), softmax runs without
max-subtraction, row-sums come from a ones-column appended to V, and the
half-head projection partials are summed on host (bf16 output DMAs, paired
two token-tiles per transfer to halve HWDGE issue serialization).

Scheduling: steady state is PE-paced (~10.7us/head); ACT runs ONLY exp
(48 x [128,1024] = the would-be pacer) plus the k-copy, injected as column
halves mid-head where PE density absorbs the ACT-queue block. Per-head
pipeline: next head's qk projection injected in chunks at kb=1,2 of the
current S/exp ladder, q-scale + k-copy at kb=3, and both rel-pos tables at
kb=5 into a SINGLE merged PSUM tile (rows 0:32 rel_h, 32:64 rel_w) so the
head boundary costs one ladder slot and S(h+1,0) issues right at exp(h,7).
The V projection fills head 0's ladder (plus B0,B1 during the load tail).
After AV(7), the attn-out and denominator rows are copied PSUM->SBUF on DVE
immediately (releases the AV accumulator before AV(h+1,0) needs the slot);
the recip -> Pool partition-broadcast -> mul normalize then runs off the
critical path. The denominator must reach partition 0 via a PLAIN copy: the
custom-DVE reciprocal mishandles nonzero input partition offsets on HW, and
partition offsets must be multiples of 32 (BIR). The last head normalizes in
column halves (denom halves via ACT, which is idle by then) so the final
projection's t=2 matmuls start on half 1; the projection prefetches two PSUM
accumulations before that. PE warm-up matmuls on zeros during the initial
DMA wait keep the p-state ramp (and the cost model's dispatch-time pricing)
at full clock, and a dummy exp preloads the ACT table.

Cost-model (TimelineSim) estimate: ~102.1us/core (baseline: 133.0us);
engine busy ~ PE 83us, ACT 62us, DVE 52us. End-to-end rel err vs the fp32
jax reference: ~4.5e-3 (bf16-dominated).
"""
from contextlib import ExitStack

import numpy as np
import ml_dtypes

import concourse.bass as bass
import concourse.bacc as bacc
import concourse.mybir as mybir
import concourse.tile as tile
from concourse.bass_utils import run_bass_kernel_spmd

B, H, W, DIM, HEADS = 4, 32, 32, 768, 12
HD = DIM // HEADS  # 64
N = H * W  # 1024
HPC = HEADS // 2  # heads per core = 6
NCORES = 8
F32 = mybir.dt.float32
BF16 = mybir.dt.bfloat16

_cache = {}


def build_program(reps=1, debug=False):
    nc = bacc.Bacc("TRN2", target_bir_lowering=False, debug=False,
                   enable_asserts=False, num_devices=NCORES)
    xtf = nc.dram_tensor("xtf", [128, 7 * N], BF16, kind="ExternalInput")
    wqkf = nc.dram_tensor("wqkf", [128, HPC * 896], BF16, kind="ExternalInput")
    wvf = nc.dram_tensor("wvf", [128, 7 * 390], BF16, kind="ExternalInput")
    wpf = nc.dram_tensor("wpf", [128, 3 * 768], BF16, kind="ExternalInput")
    tabf = nc.dram_tensor("tabf", [HD, 2 * N], BF16, kind="ExternalInput")
    ecf = nc.dram_tensor("ecf", [64, N], BF16, kind="ExternalInput")
    bqf = nc.dram_tensor("bqf", [128, HPC], mybir.dt.float32, kind="ExternalInput")
    out_d = nc.dram_tensor("out_part", [N, DIM], BF16, kind="ExternalOutput")
    dbg = None
    if debug:
        dbg = {
            "rhs20": nc.dram_tensor("dbg_rhs20", [128, N], BF16, kind="ExternalOutput"),
            "lhsT0": nc.dram_tensor("dbg_lhsT0", [128, N], BF16, kind="ExternalOutput"),
            "at0": nc.dram_tensor("dbg_at0", [128, N], BF16, kind="ExternalOutput"),
            "pav0": nc.dram_tensor("dbg_pav0", [65, N], F32, kind="ExternalOutput"),
            "v0": nc.dram_tensor("dbg_v0", [128, 390], BF16, kind="ExternalOutput"),
            "xt0": nc.dram_tensor("dbg_xt0", [128, 7 * N], BF16, kind="ExternalOutput"),
            "rb0": nc.dram_tensor("dbg_rb0", [64, N], F32, kind="ExternalOutput"),
            "pl0": nc.dram_tensor("dbg_pl0", [128, N], BF16, kind="ExternalOutput"),
            "plf0": nc.dram_tensor("dbg_plf0", [128, N], BF16, kind="ExternalOutput"),
            "plf1": nc.dram_tensor("dbg_plf1", [128, N], BF16, kind="ExternalOutput"),
            "plf2": nc.dram_tensor("dbg_plf2", [128, N], BF16, kind="ExternalOutput"),
        }

    with ExitStack() as ctx:
        tc = ctx.enter_context(tile.TileContext(nc))
        _body(nc, tc, ctx, xtf, wqkf, wvf, wpf, tabf, ecf, bqf, out_d, reps, dbg)
    nc.compile()
    return nc


def _body(nc, tc, ctx, xtf, wqkf, wvf, wpf, tabf, ecf, bqf, out_d, reps, dbg=None):
    persist = ctx.enter_context(tc.tile_pool(name="persist", bufs=1))
    comb = ctx.enter_context(tc.tile_pool(name="comb", bufs=2))
    attn_pool = ctx.enter_context(tc.tile_pool(name="attn", bufs=1))
    nav = ctx.enter_context(tc.tile_pool(name="nav", bufs=3))
    outp = ctx.enter_context(tc.tile_pool(name="outp", bufs=4))
    ps_s = ctx.enter_context(tc.tile_pool(name="ps_s", bufs=2, space="PSUM"))
    ps_o = ctx.enter_context(tc.tile_pool(name="ps_o", bufs=2, space="PSUM"))

    xt = persist.tile([128, 7 * N], BF16, tag="xt", name="xt")
    wqk_sb = [persist.tile([128, 896], BF16, tag=f"wqk{h}", name=f"wqk{h}")
              for h in range(HPC)]
    wv_sb = persist.tile([128, 7 * 390], BF16, tag="wv", name="wv")
    wp_sb = persist.tile([128, 3 * 768], BF16, tag="wp", name="wp")
    tab = persist.tile([HD, 2 * N], BF16, tag="tab", name="tab")
    rhs2 = [persist.tile([128, N], BF16, tag=f"rhs{i}", name=f"rhs{i}")
            for i in range(2)]
    bq_sb = persist.tile([128, HPC], mybir.dt.float32, tag="bq", name="bq_sb")
    v_sb = [persist.tile([128, 390], BF16, tag=f"v{m}", name=f"v{m}")
            for m in range(8)]
    plhs = [persist.tile([128, N], BF16, tag=f"pl{t}", name=f"pl{t}")
            for t in range(3)]

    # ---- input loads: one SP queue, priority order (HWDGE+DMA serialize).
    # xt in two half-column DMAs: the QK projection's first column half can
    # start after only half the x bytes have landed.
    q = nc.sync
    q.dma_start(xt[:, 0:1024], xtf[:, 0:1024])
    q.dma_start(wqk_sb[0][:], wqkf[:, 0:896])
    q.dma_start(xt[:, 1024:3072], xtf[:, 1024:3072])
    q.dma_start(xt[:, 3072:5120], xtf[:, 3072:5120])
    q.dma_start(xt[:, 5120:7168], xtf[:, 5120:7168])
    q.dma_start(tab[:], tabf[:])
    q.dma_start(bq_sb[:], bqf[:])
    q.dma_start(rhs2[0][64:128, :], ecf[:])
    q.dma_start(wv_sb[:], wvf[:])
    q.dma_start(rhs2[1][64:128, :], ecf[:])
    for h in range(1, HPC):
        q.dma_start(wqk_sb[h][:], wqkf[:, h * 896:(h + 1) * 896])
    q.dma_start(wp_sb[:], wpf[:])

    # preload the Exp activation table while DMAs stream (otherwise the
    # 1.3us table load lands right before the first real exp)
    warm = persist.tile([1, 2], F32, tag="warm", name="warm")
    nc.gpsimd.memzero(warm[:])
    nc.scalar.activation(warm[0:1, 0:1], warm[0:1, 1:2],
                         mybir.ActivationFunctionType.Exp)

    # ---- phase helpers ----
    def A_chunk(h, pqk, kcs, halves=(0, 1)):
        # qk projection for head h, contraction tiles kcs (kc=6, the bias
        # row, is skipped: biases ride the scale / k-copy as [P,1] scalars)
        for half in halves:
            sl = slice(half * 512, half * 512 + 512)
            for kc in kcs:
                nc.tensor.matmul(
                    pqk[:, sl], wqk_sb[h][:, kc * 128:(kc + 1) * 128],
                    xt[:, kc * N + half * 512: kc * N + half * 512 + 512],
                    start=(kc == 0), stop=(kc == 5))

    def A_scale(h, pqk):
        lhsT_c = comb.tile([128, N], BF16, tag="lc", name="lhsT_c")
        nc.vector.tensor_scalar(lhsT_c[0:64, :], pqk[0:64, :],
                                bq_sb[0:64, h:h + 1], 0.125,
                                mybir.AluOpType.add, mybir.AluOpType.mult)
        return lhsT_c

    def k_copy(h, pqk):
        # Pool cannot read PSUM; ACT has slack (exp-only ~8.3us vs PE ~10.7us)
        # halves: shorter ACT-queue blocks between exps. Identity + bias AP
        # adds the k projection bias (kc=6 skipped in A_chunk).
        for sl in (slice(0, 512), slice(512, 1024)):
            nc.scalar.activation(rhs2[h % 2][0:64, sl], pqk[64:128, sl],
                                 mybir.ActivationFunctionType.Identity,
                                 bias=bq_sb[64:128, h:h + 1])

    def C_mms(lhsT_c):
        # both rel tables in ONE PSUM tile (rows 0:32 rel_h, 32:64 rel_w) so
        # C(h+1) consumes a single ladder slot at the head boundary
        pc = ps_s.tile([64, N], F32, tag="s", name="pc")
        for qh in range(32):
            sl = slice(qh * 32, qh * 32 + 32)
            nc.tensor.matmul(pc[0:32, sl], tab[:, sl], lhsT_c[0:64, sl],
                             start=True, stop=True)
        qT3 = lhsT_c[0:64, :].rearrange("p (a b) -> p b a", b=32)
        for qw in range(32):
            sl = slice(qw * 32, qw * 32 + 32)
            nc.tensor.matmul(pc[32:64, sl], tab[:, N + qw * 32:N + qw * 32 + 32],
                             qT3[:, qw, :], start=True, stop=True)
        return pc

    def C_copies(lhsT_c, pc, par=False):
        # half-1 copies first: they gate S(h,0); slot release gates S(h,1)
        prw_v = pc[32:64, :].rearrange("p (a b) -> p b a", b=32)
        # par: ACT takes the rel_h copies (only safe while ACT is exp-free)
        e1 = nc.scalar if par else nc.vector
        (e1.copy if par else e1.tensor_copy)(lhsT_c[64:96, 0:512], pc[0:32, 0:512])
        nc.vector.tensor_copy(lhsT_c[96:128, 0:512], prw_v[:, 0:16, :])
        (e1.copy if par else e1.tensor_copy)(lhsT_c[64:96, 512:1024], pc[0:32, 512:1024])
        nc.vector.tensor_copy(lhsT_c[96:128, 512:1024], prw_v[:, 16:32, :])

    def B_unit(m):
        pv = ps_s.tile([128, 390], F32, tag="s", name="pv")
        for kc in range(7):
            nc.tensor.matmul(pv[:], xt[:, kc * N + m * 128: kc * N + (m + 1) * 128],
                             wv_sb[:, kc * 390:(kc + 1) * 390],
                             start=(kc == 0), stop=(kc == 6))
        nc.vector.tensor_copy(v_sb[m][:], pv[:])

    # ---- D ladder ----
    def D_phase(h, lhsT_c, extra, post_s7):
        po = ps_o.tile([65, N], F32, tag="o", name="po")
        attnT = [attn_pool.tile([128, N], BF16, tag=f"at{kb}", name=f"at{kb}")
                 for kb in range(8)]

        def S_unit(kb, split_exp=False):
            ps = ps_s.tile([128, N], F32, tag="s", name="s_ps")
            for half in range(2):
                sl = slice(half * 512, half * 512 + 512)
                nc.tensor.matmul(ps[:, sl], rhs2[h % 2][:, kb * 128:(kb + 1) * 128],
                                 lhsT_c[:, sl], start=True, stop=True)
            if split_exp:
                # halves: AV(kb)-h1 and the normalize h1 chain start earlier
                for half in range(2):
                    sl = slice(half * 512, half * 512 + 512)
                    nc.scalar.activation(attnT[kb][:, sl], ps[:, sl],
                                         mybir.ActivationFunctionType.Exp)
            else:
                nc.scalar.activation(attnT[kb][:], ps[:],
                                     mybir.ActivationFunctionType.Exp)

        def AV_unit(kb):
            for half in range(2):
                sl = slice(half * 512, half * 512 + 512)
                nc.tensor.matmul(po[:, sl], v_sb[kb][:, h * 65:(h + 1) * 65],
                                 attnT[kb][:, sl], start=(kb == 0), stop=(kb == 7))

        for kb in range(8):
            S_unit(kb)
            if dbg is not None and h == 0 and kb == 0:
                nc.sync.dma_start(dbg["at0"][:], attnT[0][:])
            if dbg is not None and h == 0 and kb == 3:
                nc.sync.dma_start(dbg["v0"][:], v_sb[0][:])
            if kb in extra:
                for f in extra[kb]:
                    f()
            if kb >= 3:
                AV_unit(kb - 3)
        if post_s7 is not None:
            post_s7()

        def tail(last=False):
            AV_unit(5)
            AV_unit(6)
            AV_unit(7)
            t = plhs[h // 2][(h % 2) * 64:(h % 2) * 64 + 64, :]
            # po row 64 = softmax denominator (ones-column of V), rows 0:64 =
            # attention output. Copy PSUM->SBUF promptly to free the AV
            # accumulator; normalize runs off the critical path. The denom
            # row must reach partition 0 via a PLAIN copy (the custom-DVE
            # reciprocal mishandles nonzero input partition offsets on HW).
            if not last:
                dn = nav.tile([1, N], F32, tag="dn", name="dn")
                nc.vector.tensor_copy(dn[:], po[64:65, :])
                pav = nav.tile([64, N], F32, tag="pav", name="pav")
                nc.vector.tensor_copy(pav[:], po[0:64, :])
                if dbg is not None and h == 0:
                    nc.sync.dma_start(dbg["pav0"][:], pav[:])
                recip = nav.tile([1, N], F32, tag="rc", name="recip")
                nc.vector.reciprocal_approx_fast(out=recip[:], in_=dn[:])
                rb = nav.tile([64, N], F32, tag="rb", name="rb")
                nc.gpsimd.partition_broadcast(rb[:], recip[:])
                nc.vector.tensor_mul(t[:], pav[:], rb[:])
                if dbg is not None and h == 0:
                    nc.sync.dma_start(dbg["rb0"][:], rb[:])
                    nc.sync.dma_start(dbg["pl0"][:], plhs[0][:])
            else:
                # last head: denom row via ACT (free now); one full pav copy
                # releases po promptly (it gates the projection PSUM slots);
                # recip/bcast/mul per column half so t=2 matmuls start early
                # fully halved: ACT copies the denom halves, DVE runs
                # [recip-h, pav-h, mul-h] per half so mul-h1 (which gates the
                # projection's t=2 matmuls) completes as early as possible
                dn = nav.tile([1, N], F32, tag="dn", name="dn")
                pav = nav.tile([64, N], F32, tag="pav", name="pav")
                for half in range(2):
                    sl = slice(half * 512, half * 512 + 512)
                    nc.scalar.copy(dn[:, sl], po[64:65, sl])
                    recip = nav.tile([1, 512], F32, tag=f"rch{half}", name="reciph")
                    nc.vector.reciprocal_approx_fast(out=recip[:], in_=dn[:, sl])
                    rb = nav.tile([64, 512], F32, tag=f"rbh{half}", name="rbh")
                    nc.gpsimd.partition_broadcast(rb[:], recip[:])
                    nc.vector.tensor_copy(pav[:, sl], po[0:64, sl])
                    nc.vector.tensor_mul(t[:, sl], pav[:, sl], rb[:])
        return tail

    # ---- phase E: projection ----
    pps = {}

    def proj_alloc(m):
        pool, tg = (ps_s, "s") if m % 2 == 0 else (ps_o, "o")
        pps[m] = pool.tile([128, 768], F32, tag=tg, name="pp")

    def proj_mms(m, ts):
        pp = pps[m]
        for t in ts:
            for n0, nw in ((0, 512), (512, 256)):
                nc.tensor.matmul(pp[:, n0:n0 + nw],
                                 plhs[t][:, m * 128:(m + 1) * 128],
                                 wp_sb[:, t * 768 + n0: t * 768 + n0 + nw],
                                 start=(t == 0), stop=(t == 2))

    osb_pair = {}

    def proj_out(m):
        # m 0..5: copies per m (ACT/DVE alternating) into a paired staging
        # tile, ONE DMA per (even, odd) pair — halves serialized HWDGE issue.
        # m 6,7: single DMAs so the final chain after the last matmul is short
        if m >= 6:
            osb = outp.tile([128, DIM], BF16, tag="osbs", name="osbs")
            if m % 2 == 0:
                nc.vector.tensor_copy(osb[:], pps[m][:])
            else:
                nc.scalar.copy(osb[:], pps[m][:])
            eng = nc.sync if m % 2 == 0 else nc.scalar
            eng.dma_start(out_d[m * 128:(m + 1) * 128, :], osb[:])
        elif m % 2 == 0:
            osb = outp.tile([128, 2 * DIM], BF16, tag="osb", name="osb")
            osb_pair[m // 2] = osb
            nc.vector.tensor_copy(osb[:, 0:DIM], pps[m][:])
        else:
            osb = osb_pair[m // 2]
            nc.scalar.copy(osb[:, DIM:2 * DIM], pps[m][:])
            dview = out_d[(m - 1) * 128:(m + 1) * 128, :].rearrange(
                "(j p) d -> p j d", j=2)
            sview = osb[:].rearrange("p (j d) -> p j d", j=2)
            eng = nc.sync if m % 4 == 1 else nc.scalar
            eng.dma_start(dview, sview)

    # ---- main schedule ----
    for _rep in range(reps):
        pqk = ps_o.tile([128, N], F32, tag="o", name="pqk")
        if _rep == 0:
            # warm-up matmuls on zeros while input DMAs stream: the PE
            # p-state ramps with continuous execution, so the first real
            # matmuls are priced at full clock. pqk is reset by A's start=True.
            wu = persist.tile([64, 512], BF16, tag="wu", name="wu")
            nc.vector.memzero(wu[:])
            for _ in range(8):
                nc.tensor.matmul(pqk[0:64, 0:512], wu[:, 0:64], wu[:],
                                 start=True, stop=True)
        A_chunk(0, pqk, range(6))
        lhsT_c = A_scale(0, pqk)
        k_copy(0, pqk)
        pc0 = C_mms(lhsT_c)
        if _rep == 0:
            B_unit(0)  # fills PE while C-copies run on DVE
            B_unit(1)
        C_copies(lhsT_c, pc0)
        if dbg is not None:
            nc.sync.dma_start(dbg["xt0"][:], xt[:])
            nc.sync.dma_start(dbg["rhs20"][:], rhs2[0][:])
            nc.sync.dma_start(dbg["lhsT0"][:], lhsT_c[:])

        nxt = {}
        for h in range(HPC):
            extra = {}
            if h + 1 < HPC:
                def inj_a1(h=h):
                    nxt["pqk"] = ps_o.tile([128, N], F32, tag="o", name="pqk")
                    A_chunk(h + 1, nxt["pqk"], range(0, 3))

                def inj_a2(h=h):
                    A_chunk(h + 1, nxt["pqk"], range(3, 6))

                def inj_scale(h=h):
                    nxt["lhsT_c"] = A_scale(h + 1, nxt["pqk"])
                    k_copy(h + 1, nxt["pqk"])

                def inj_c(h=h):
                    pc = C_mms(nxt["lhsT_c"])
                    C_copies(nxt["lhsT_c"], pc)

                extra = {1: [inj_a1], 2: [inj_a2], 3: [inj_scale], 5: [inj_c]}
            if h == 0 and _rep == 0:
                base = dict(extra)
                for kb in range(6):
                    def mk(kb=kb):
                        return lambda: B_unit(kb + 2)
                    extra.setdefault(kb, [])
                    if kb in base:
                        extra[kb] = list(base[kb]) + [mk()]
                    else:
                        extra[kb] = [mk()]

            post_s7 = None
            if h == HPC - 1 and _rep == reps - 1:
                def post_s7():
                    proj_alloc(0)
                    proj_mms(0, (0, 1))
                    proj_alloc(1)
                    proj_mms(1, (0, 1))

            tail = D_phase(h, lhsT_c, extra, post_s7)
            if h + 1 < HPC:
                tail()
                lhsT_c = nxt["lhsT_c"]
            else:
                tail(last=True)

    if dbg is not None:
        for t in range(3):
            nc.sync.dma_start(dbg[f"plf{t}"][:], plhs[t][:])
    # E tail: t=2 for prefetched m, then the rest
    proj_alloc(2)
    proj_mms(2, (0, 1))
    for m in range(3):
        proj_mms(m, (2,))
        proj_out(m)
    proj_alloc(3)
    proj_mms(3, (0, 1))
    proj_mms(3, (2,))
    proj_out(3)
    for m in range(4, 8):
        proj_alloc(m)
        proj_mms(m, (0, 1, 2))
        proj_out(m)


def _host_prep(x, qkv_w, qkv_b, proj_w, proj_b, rel_pos_h, rel_pos_w):
    bf16 = ml_dtypes.bfloat16
    idx_h = np.arange(H)[:, None] - np.arange(H)[None, :] + (H - 1)
    idx_w = np.arange(W)[:, None] - np.arange(W)[None, :] + (W - 1)
    Rh = rel_pos_h[idx_h]  # [qh, kh, c]
    Rw = rel_pos_w[idx_w]  # [qw, kw, c]
    rhT8 = (8.0 * Rh).transpose(2, 0, 1).reshape(HD, H * H)
    rwT8 = (8.0 * Rw).transpose(2, 0, 1).reshape(HD, W * W)
    tabf = np.ascontiguousarray(
        np.concatenate([rhT8, rwT8], axis=1)).astype(bf16)
    kt = np.arange(N)
    ec = np.zeros((64, N), np.float32)
    ec[:32] = (np.arange(32)[:, None] == (kt // 32)[None, :])
    ec[32:] = (np.arange(32)[:, None] == (kt % 32)[None, :])
    ecf = ec.astype(bf16)

    in_maps = []
    for core in range(NCORES):
        b = core // 2
        h0 = (core % 2) * HPC
        xT = x[b].reshape(N, DIM).T  # [768, 1024]
        xtf = np.zeros((128, 7 * N), np.float32)
        for kc in range(6):
            xtf[:, kc * N:(kc + 1) * N] = xT[kc * 128:(kc + 1) * 128]
        xtf[0, 6 * N:7 * N] = 1.0

        wqkf = np.zeros((128, HPC * 896), np.float32)
        wvx = np.zeros((769, 7 * 0 + 390), np.float32)
        wpm = np.zeros((HPC * HD, DIM), np.float32)
        for h in range(HPC):
            g = h0 + h
            blk = np.zeros((769, 128), np.float32)
            blk[:DIM, 0:64] = qkv_w[g * HD:(g + 1) * HD].T
            blk[DIM, 0:64] = qkv_b[g * HD:(g + 1) * HD]
            blk[:DIM, 64:128] = qkv_w[DIM + g * HD:DIM + (g + 1) * HD].T
            blk[DIM, 64:128] = qkv_b[DIM + g * HD:DIM + (g + 1) * HD]
            for kc in range(6):
                wqkf[:, h * 896 + kc * 128: h * 896 + (kc + 1) * 128] = \
                    blk[kc * 128:(kc + 1) * 128]
            wqkf[0, h * 896 + 768: h * 896 + 896] = blk[DIM]

            wvx[:DIM, h * 65:h * 65 + 64] = \
                qkv_w[2 * DIM + g * HD:2 * DIM + (g + 1) * HD].T
            wvx[DIM, h * 65:h * 65 + 64] = \
                qkv_b[2 * DIM + g * HD:2 * DIM + (g + 1) * HD]
            wvx[DIM, h * 65 + 64] = 1.0
            wpm[h * HD:(h + 1) * HD, :] = proj_w[:, g * HD:(g + 1) * HD].T

        wvf = np.zeros((128, 7 * 390), np.float32)
        for kc in range(6):
            wvf[:, kc * 390:(kc + 1) * 390] = wvx[kc * 128:(kc + 1) * 128]
        wvf[0, 6 * 390:7 * 390] = wvx[DIM]

        wpf = np.zeros((128, 3 * 768), np.float32)
        for t in range(3):
            wpf[:, t * 768:(t + 1) * 768] = wpm[t * 128:(t + 1) * 128]

        bqm = np.zeros((128, HPC), np.float32)
        for h in range(HPC):
            g = h0 + h
            bqm[0:64, h] = qkv_b[g * HD:(g + 1) * HD]
            bqm[64:128, h] = qkv_b[DIM + g * HD:DIM + (g + 1) * HD]
        in_maps.append({
            "xtf": xtf.astype(bf16), "wqkf": wqkf.astype(bf16),
            "wvf": wvf.astype(bf16), "wpf": wpf.astype(bf16),
            "tabf": tabf, "ecf": ecf, "bqf": bqm,
        })
    return in_maps


def kernel(x, qkv_w, qkv_b, proj_w, proj_b, rel_pos_h, rel_pos_w, _trace=False):
    x = np.asarray(x, np.float32)
    qkv_w = np.asarray(qkv_w, np.float32)
    qkv_b = np.asarray(qkv_b, np.float32)
    proj_w = np.asarray(proj_w, np.float32)
    proj_b = np.asarray(proj_b, np.float32)
    rel_pos_h = np.asarray(rel_pos_h, np.float32)
    rel_pos_w = np.asarray(rel_pos_w, np.float32)

    in_maps = _host_prep(x, qkv_w, qkv_b, proj_w, proj_b, rel_pos_h, rel_pos_w)
    if "nc" not in _cache:
        _cache["nc"] = build_program()
    nc = _cache["nc"]
    res = run_bass_kernel_spmd(nc, in_maps, core_ids=list(range(NCORES)),
                               trace=_trace)
    parts = [r["out_part"].astype(np.float32) for r in res.results]
    out = np.zeros((B, N, DIM), np.float32)
    for b in range(B):
        out[b] = parts[2 * b] + parts[2 * b + 1] + proj_b
    if _trace:
        kernel.last_results = res
    return out.reshape(B, H, W, DIM)
